# revision 46
# baseline (speedup 1.0000x reference)
"""Trainium2 Bass kernel: sampled logistic-regression forward.

reference math (per data row i, sample s):
    mean_i = X[i] . w_mu
    var_i  = sum_d X[i,d]^2 * exp(w_log_var[d])
    out[i,s] = sigmoid( sqrt(var_i) * z[s] + mean_i )

Full shapes: X [500000, 64], w_mu [64], w_log_var [64], z [128]
Output: [500000, 128] fp32.

Sharding: data-parallel over 8 NeuronCores, 62500 rows each.

Layout: chunk-local stripe. Chunk c covers shard rows
[base, base + 125*TC) -- one contiguous DRAM range per DMA -- and
within the chunk, partition p holds rows base + p*TC + t. Each DMA
descriptor is a per-partition contiguous run of TC rows and each DMA
instruction's DRAM side is one contiguous range. DMAs split into a
120-descriptor + 5-descriptor instruction pair: descriptor counts
divisible by 15 fan out across all 15 DMA engines.

Numerics: tolerance is rel 2e-2; measured full-size error is ~5.4e-3:
  - stats are plain f32r (11-bit mantissa) [mean, std] rows (K=2), no
    hi/lo splitting or mantissa masks;
  - X rides the sync ring, whose E69+ DMA engines round f32 payloads
    to f32r (~+1e-3);
  - the output is written to DRAM as bf16 (sigmoid in [0,1] -> <1e-3
    absolute) and upcast to f32 on the host, halving write traffic.
All three simplifications were validated against the float64 reference
on the actual seed-0 data before being adopted.

Per-core pipeline, chunks of up to SC=48 tiles x [125 rows, 64],
loaded in PAIRS (one DMA covers two chunks: 24 KB descriptors, half
the issue instructions; outputs use the pair's tw-stripe):
  - in-DMA pair (sync ring, issued 2 pairs ahead from the otherwise
    idle sync sequencer; target slot freed a pair ago so the issue
    never parks the ring on a buffer WAR)
  - ACT: X2 = Square(X) (Square lives in the sigmoid table set)
  - DVE/GPSIMD: A = X * w_mu split ~60/40 (stride-0 broadcast of the
    [P, 1, D] weight rows -- measured same speed as materialized reps);
    V = X2 * exp(lv) in place on GPSIMD
  - DVE: reduce A -> statblk[:, :, 0] (mean); reduce V -> var
  - DVE+GPSIMD: Newton rsqrt (bit-trick seed, 2 iters);
    statblk[:, :, 1] = var * y (std)
  - PE: transpose statblk [125, (t k)] -> [2*TC, 125]; ACT copy ->
    SBUF f32r (rows 2t/2t+1 = tile t's mean/std, so each 4-tile
    group's 8 stat rows are contiguous partitions)
  - PE: per 4-tile group one matmul lhsT=s2[b:b+K] (K<=32, base
    partition b in {0,32,64}) x rhs=z2g[b:b+K, 512-col variant] where
    z2g [96, 2048] f32r holds 3 stacked 32-row blocks x 4 column
    variants of the block-diagonal [ones; z] pattern
  - ACT: Sigmoid [125, 1024] over PSUM bank pairs -> outb (bf16)
  - out-DMA full chunk (scalar ring -- the sync ring would corrupt
    bf16 payloads by f32r word-masking)
"""

from contextlib import ExitStack

import numpy as np

import concourse.bacc as bacc
import concourse.bass as bass
import concourse.tile as tile
from concourse import mybir
from concourse.bass_utils import run_bass_kernel_spmd

N_CORES = 8
D = 64
NS = 128
P = 125          # rows per tile (partition dim)
SC = 48          # max tiles per chunk (DMA + stats granularity)
GT = 4           # tiles per matmul group (4*128 = 512 f32 = 1 PSUM bank)
PSPLIT = 120     # descriptor-count split: 120 (15 engines) + 5

RSQRT_MAGIC = 0x5F3759DF
F32 = mybir.dt.float32
F32R = mybir.dt.float32r
BF16 = mybir.dt.bfloat16
U32 = mybir.dt.uint32


def _split_dma(nc, out_ap, in_ap, eng):
    eng.dma_start(out=out_ap[0:PSPLIT], in_=in_ap[0:PSPLIT])
    eng.dma_start(out=out_ap[PSPLIT:P], in_=in_ap[PSPLIT:P])


def _schedule(ntiles: int) -> list[int]:
    """Graded chunk sizes: small head (fast pipeline fill), 48s in the
    middle, tapered tail (short drain). All sizes are multiples of 4."""
    sched = []
    rem = ntiles
    for s in (8, 8, 16, 32):
        if rem >= s + SC:
            sched.append(s)
            rem -= s
    while rem > SC + 8:
        sched.append(SC)
        rem -= SC
    if rem > 8:
        sched.append(rem - 8)
        sched.append(8)
    elif rem:
        sched.append(rem)
    assert sum(sched) == ntiles, (sched, ntiles)
    assert all(s <= SC and s % 4 == 0 for s in sched), sched
    return sched


def build_program(rows: int):
    """Build the single-core Bass/Tile program for `rows` rows (SPMD across cores)."""
    assert rows % P == 0
    ntiles = rows // P
    assert ntiles % GT == 0

    nc = bacc.Bacc(
        "TRN2",
        target_bir_lowering=False,
        debug=False,
        num_devices=N_CORES,
    )

    x = nc.dram_tensor("x", [rows, D], F32, kind="ExternalInput")
    wmu_d = nc.dram_tensor("wmu", [P, D], F32, kind="ExternalInput")
    elv_d = nc.dram_tensor("elv", [P, D], F32, kind="ExternalInput")
    z2g_d = nc.dram_tensor("z2g", [96, 4 * GT * NS], F32R, kind="ExternalInput")
    ident = nc.dram_tensor("ident", [P, P], F32, kind="ExternalInput")
    # bf16 output: sigmoid values lie in [0, 1], so bf16 costs <1e-3
    # absolute error; the host upcasts to f32 after the gather. This
    # halves the HBM write traffic (the dominant stream).
    out = nc.dram_tensor("out", [rows, NS], BF16, kind="ExternalOutput")

    sched = _schedule(ntiles)
    bases = []
    c0 = 0
    for TC in sched:
        bases.append(c0)
        c0 += TC

    with tile.TileContext(nc) as tc, ExitStack() as ctx:
        singles = ctx.enter_context(tc.tile_pool(name="singles", bufs=1))
        xin = ctx.enter_context(tc.tile_pool(name="xin", bufs=3))
        sqp = ctx.enter_context(tc.tile_pool(name="sqp", bufs=3))
        amp = ctx.enter_context(tc.tile_pool(name="amp", bufs=3))
        statp = ctx.enter_context(tc.tile_pool(name="statp", bufs=4))
        smalls = ctx.enter_context(tc.tile_pool(name="smalls", bufs=4))
        s2p = ctx.enter_context(tc.tile_pool(name="s2p", bufs=4))
        outp = ctx.enter_context(tc.tile_pool(name="outp", bufs=4))
        pst_pool = ctx.enter_context(tc.tile_pool(name="pst", bufs=2, space="PSUM"))
        paff_pool = ctx.enter_context(tc.tile_pool(name="paff", bufs=3, space="PSUM"))

        # chunks are loaded in PAIRS: one DMA covers two consecutive
        # chunks (bigger descriptors, half the issue instructions)
        pairs = []
        i = 0
        while i < len(sched):
            if i + 1 < len(sched):
                pairs.append((i, i + 1))
                i += 2
            else:
                pairs.append((i, None))
                i += 1

        xts = {}

        def issue_pair(pi):
            ca, cb = pairs[pi]
            tw = sched[ca] + (sched[cb] if cb is not None else 0)
            row0 = bases[ca] * P
            xc = x[row0 : row0 + P * tw, :].rearrange("(p t) d -> p t d", p=P)
            # sync ring: E69+ round f32 payloads to f32r there, which
            # costs only ~1e-3 extra output error (host-validated); in
            # exchange the input stream is issued from the otherwise-idle
            # sync sequencer and the scalar ring is free for the bf16
            # outputs (which the sync ring would corrupt by word-masking)
            xt = xin.tile([P, 2 * SC, D], F32, name="xt")
            _split_dma(nc, xt[:, :tw, :], xc, eng=nc.sync)
            # NOTE: the pair is striped as ONE tw-tile chunk (partition p
            # holds rows row0 + p*tw + t); each chunk's output must use
            # the same striping, so carry (row0, tw, toff) along.
            xts[ca] = (xt, 0, row0, tw)
            if cb is not None:
                xts[cb] = (xt, sched[ca], row0, tw)

        issue_pair(0)

        # ---- one-time consts ----
        wmu_p = singles.tile([P, 1, D], F32)
        nc.scalar.dma_start(out=wmu_p[:, 0, :], in_=wmu_d[:, :])
        elv_p = singles.tile([P, 1, D], F32)
        nc.scalar.dma_start(out=elv_p[:, 0, :], in_=elv_d[:, :])
        z2g_sb = singles.tile([96, 4 * GT * NS], F32R)
        nc.sync.dma_start(out=z2g_sb, in_=z2g_d[:, :])
        id_sb = singles.tile([P, P], F32)
        nc.sync.dma_start(out=id_sb, in_=ident[:, :])
        magic_sb = singles.tile([P, SC], U32)
        nc.vector.memset(magic_sb, RSQRT_MAGIC)
        one_sb = singles.tile([P, 1], U32)
        nc.vector.memset(one_sb, 1)

        # ---- chunk pipeline ----
        for pi0 in (1, 2):
            if pi0 < len(pairs):
                issue_pair(pi0)

        first_of_pair = {ca: pi for pi, (ca, cb) in enumerate(pairs)}

        for ci, TC in enumerate(sched):
            xtp, toff, prow0, ptw = xts.pop(ci)
            oc = out[prow0 : prow0 + P * ptw, :].rearrange(
                "(p t) s -> p t s", p=P
            )[:, toff : toff + TC, :]
            xt = xtp[:, toff : toff + TC, :]

            # X^2 on ACT (Square lives in the sigmoid table set)
            x2 = sqp.tile([P, SC, D], F32)
            nc.scalar.activation(
                out=x2[:, :TC, :], in_=xt,
                func=mybir.ActivationFunctionType.Square,
            )
            # prefetch two pairs ahead at each pair's first chunk; the
            # target slot was freed a pair ago, so the dma_start never
            # blocks its ring on the buffer WAR
            pi = first_of_pair.get(ci)
            if pi is not None and pi >= 1 and pi + 2 < len(pairs):
                issue_pair(pi + 2)

            # A = X * w_mu (stride-0 broadcast over t), split ~60/40
            # DVE/GPSIMD; V = X^2 * exp(lv) in place on GPSIMD.
            # Measured rates: DVE big-mul ~77, GPSIMD ~61 G elem/s;
            # reduces+NR put ~6.8us/chunk of fixed work on DVE.
            at = amp.tile([P, SC, D], F32)
            AS = max(4, (3 * TC // 5) & ~3)
            nc.vector.tensor_mul(
                at[:, :AS, :], xt[:, :AS, :], wmu_p.to_broadcast((P, AS, D))
            )
            nc.gpsimd.tensor_mul(
                at[:, AS:TC, :], xt[:, AS:TC, :],
                wmu_p.to_broadcast((P, TC - AS, D)),
            )
            nc.gpsimd.tensor_mul(
                x2[:, :TC, :], x2[:, :TC, :], elv_p.to_broadcast((P, TC, D))
            )

            statblk = statp.tile([P, SC, 2], F32)
            nc.vector.tensor_reduce(
                out=statblk[:, :TC, 0],
                in_=at[:, :TC, :],
                axis=mybir.AxisListType.X,
                op=mybir.AluOpType.add,
            )
            var = smalls.tile([P, SC], F32)
            nc.vector.tensor_reduce(
                out=var[:, :TC],
                in_=x2[:, :TC, :],
                axis=mybir.AxisListType.X,
                op=mybir.AluOpType.add,
            )

            # y = rsqrt(var): seed 0x5f3759df - (bits >> 1), 2 NR iters
            vb = var[:, :TC].bitcast(U32)
            yb = smalls.tile([P, SC], U32)
            nc.vector.tensor_scalar(
                yb[:, :TC], vb, one_sb[:, 0:1], None,
                op0=mybir.AluOpType.logical_shift_right,
            )
            nc.vector.scalar_tensor_tensor(
                out=yb[:, :TC],
                in0=magic_sb[:, :TC],
                scalar=0,
                in1=yb[:, :TC],
                op0=mybir.AluOpType.bypass,
                op1=mybir.AluOpType.subtract,
            )
            y = yb.bitcast(F32)
            t2 = smalls.tile([P, SC], F32)
            for _ in range(2):
                # y <- y*(1.5 - 0.5*var*y^2)
                nc.gpsimd.tensor_mul(t2[:, :TC], y[:, :TC], y[:, :TC])
                nc.vector.scalar_tensor_tensor(
                    out=t2[:, :TC], in0=t2[:, :TC], scalar=-0.5, in1=var[:, :TC],
                    op0=mybir.AluOpType.mult, op1=mybir.AluOpType.mult,
                )
                nc.vector.scalar_tensor_tensor(
                    out=y[:, :TC], in0=t2[:, :TC], scalar=1.5, in1=y[:, :TC],
                    op0=mybir.AluOpType.add, op1=mybir.AluOpType.mult,
                )
            # std -> statblk row 1
            nc.gpsimd.tensor_mul(statblk[:, :TC, 1], var[:, :TC], y[:, :TC])

            # transpose stats: [125, (t k)] -> [(t k), 125]; rows 2t/2t+1
            # hold tile t's mean/std, so group g's rows are partitions
            # [8g, 8g+8)
            pst = pst_pool.tile([2 * SC, P], F32)
            nc.tensor.transpose(
                out=pst[: 2 * TC, :],
                in_=statblk[:, :TC, :].rearrange("p t k -> p (t k)"),
                identity=id_sb,
            )
            s2 = s2p.tile([2 * SC, P], F32R)
            nc.scalar.copy(out=s2[: 2 * TC, :], in_=pst[: 2 * TC, :])

            outb = outp.tile([P, SC, NS], BF16)
            g0 = 0
            while g0 < TC:
                gw = min(2 * GT, TC - g0)          # 8 or tail 4 tiles
                pa = paff_pool.tile([P, 2, GT * NS], F32)
                for k in range(gw // GT):
                    r0 = 2 * (g0 + k * GT)     # stat-row offset of this group
                    b = (r0 // 32) * 32        # legal PE base partition
                    v = (r0 - b) // 8          # which column-variant of z2g
                    kk = min(32, 2 * TC - b)
                    nc.tensor.matmul(
                        pa[:, k, :],
                        lhsT=s2[b : b + kk, :],
                        rhs=z2g_sb[b : b + kk, v * GT * NS : (v + 1) * GT * NS],
                        start=True,
                        stop=True,
                    )
                nc.scalar.activation(
                    out=outb[:, g0 : g0 + gw, :].rearrange("p t s -> p (t s)"),
                    in_=pa.rearrange("p a b -> p (a b)")[:, : gw * NS],
                    func=mybir.ActivationFunctionType.Sigmoid,
                )
                g0 += gw

            _split_dma(nc, oc, outb[:, :TC, :], eng=nc.scalar)

    nc.finalize()
    return nc


def _rn_f32r(x: np.ndarray) -> np.ndarray:
    """Round-to-nearest-even to 11 explicit mantissa bits (f32r)."""
    u = np.ascontiguousarray(x, dtype=np.float32).view(np.uint32)
    add = np.uint32(0x800) - np.uint32(1) + ((u >> np.uint32(12)) & np.uint32(1))
    return ((u + add) & np.uint32(0xFFFFF000)).view(np.float32)


def _host_consts(w_mu: np.ndarray, w_log_var: np.ndarray, z: np.ndarray):
    elv = np.exp(np.asarray(w_log_var, dtype=np.float64)).astype(np.float32)
    z = _rn_f32r(np.asarray(z, dtype=np.float32))
    # [96, 2048]: 3 stacked 32-row blocks (PE base partitions 0/32/64),
    # each with 4 column-variants selecting which 8 rows carry the
    # block-diagonal [ones; z] pattern for a 4-tile matmul group.
    z2g = np.zeros((96, 4 * GT * NS), dtype=np.float32)
    for b in (0, 32, 64):
        for v in range(4):
            for tl in range(GT):
                r = b + 8 * v + 2 * tl
                c = v * GT * NS + tl * NS
                z2g[r, c : c + NS] = 1.0
                z2g[r + 1, c : c + NS] = z
    return {
        "wmu": np.tile(np.asarray(w_mu, dtype=np.float32)[None, :], (P, 1)),
        "elv": np.tile(elv[None, :], (P, 1)),
        "z2g": z2g,
        "ident": np.eye(P, dtype=np.float32),
    }


_PROGRAM_CACHE: dict[int, "bass.Bass"] = {}


def run(X, w_mu, w_log_var, z, trace=False):
    X = np.ascontiguousarray(X, dtype=np.float32)
    n = X.shape[0]
    assert n % N_CORES == 0
    rows = n // N_CORES
    if rows not in _PROGRAM_CACHE:
        _PROGRAM_CACHE[rows] = build_program(rows)
    nc = _PROGRAM_CACHE[rows]

    consts = _host_consts(np.asarray(w_mu), np.asarray(w_log_var), np.asarray(z))
    in_maps = [
        {"x": X[i * rows : (i + 1) * rows], **consts} for i in range(N_CORES)
    ]
    res = run_bass_kernel_spmd(nc, in_maps, list(range(N_CORES)), trace=trace)
    outs = [np.asarray(res.results[i]["out"]).astype(np.float32) for i in range(N_CORES)]
    full = np.concatenate(outs, axis=0)
    return full, res


def kernel(X, w_mu, w_log_var, z):
    full, _ = run(X, w_mu, w_log_var, z, trace=False)
    return full


# revision 47
# speedup vs baseline: 1.1803x; 1.1803x over previous
"""Trainium2 Bass kernel: sampled logistic-regression forward.

reference math (per data row i, sample s):
    mean_i = X[i] . w_mu
    var_i  = sum_d X[i,d]^2 * exp(w_log_var[d])
    out[i,s] = sigmoid( sqrt(var_i) * z[s] + mean_i )

Full shapes: X [500000, 64], w_mu [64], w_log_var [64], z [128]
Output: [500000, 128] fp32.

Sharding: data-parallel over 8 NeuronCores, 62500 rows each.

Layout: chunk-local stripe. Chunk c covers shard rows
[base, base + 125*TC) -- one contiguous DRAM range per DMA -- and
within the chunk, partition p holds rows base + p*TC + t. Each DMA
descriptor is a per-partition contiguous run of TC rows and each DMA
instruction's DRAM side is one contiguous range. DMAs split into a
120-descriptor + 5-descriptor instruction pair: descriptor counts
divisible by 15 fan out across all 15 DMA engines.

Numerics: tolerance is rel 2e-2; measured full-size error is ~5.4e-3:
  - stats are plain f32r (11-bit mantissa) [mean, std] rows (K=2), no
    hi/lo splitting or mantissa masks;
  - X rides the sync ring, whose E69+ DMA engines round f32 payloads
    to f32r (~+1e-3);
  - the output is written to DRAM as bf16 (sigmoid in [0,1] -> <1e-3
    absolute) and upcast to f32 on the host, halving write traffic.
All three simplifications were validated against the float64 reference
on the actual seed-0 data before being adopted.

Per-core pipeline, chunks of up to SC=48 tiles x [125 rows, 64],
loaded in PAIRS (one DMA covers two chunks: 24 KB descriptors, half
the issue instructions; outputs use the pair's tw-stripe):
  - in-DMA pair (sync ring, issued 2 pairs ahead from the otherwise
    idle sync sequencer; target slot freed a pair ago so the issue
    never parks the ring on a buffer WAR)
  - ACT: X2 = Square(X) (Square lives in the sigmoid table set)
  - DVE/GPSIMD: A = X * w_mu split ~60/40 (stride-0 broadcast of the
    [P, 1, D] weight rows -- measured same speed as materialized reps);
    V = X2 * exp(lv) in place on GPSIMD
  - DVE: reduce A -> statblk[:, :, 0] (mean); reduce V -> var
  - DVE+GPSIMD: Newton rsqrt (bit-trick seed, 2 iters);
    statblk[:, :, 1] = var * y (std)
  - PE: transpose statblk [125, (t k)] -> [2*TC, 125]; ACT copy ->
    SBUF f32r (rows 2t/2t+1 = tile t's mean/std, so each 4-tile
    group's 8 stat rows are contiguous partitions)
  - PE: per 4-tile group one matmul lhsT=s2[b:b+K] (K<=32, base
    partition b in {0,32,64}) x rhs=z2g[b:b+K, 512-col variant] where
    z2g [96, 2048] f32r holds 3 stacked 32-row blocks x 4 column
    variants of the block-diagonal [ones; z] pattern
  - ACT: Sigmoid [125, 1024] over PSUM bank pairs -> outb (bf16)
  - out-DMA full chunk (scalar ring -- the sync ring would corrupt
    bf16 payloads by f32r word-masking)
"""

from contextlib import ExitStack

import numpy as np

import concourse.bacc as bacc
import concourse.bass as bass
import concourse.tile as tile
from concourse import mybir
from concourse.bass_utils import run_bass_kernel_spmd

N_CORES = 8
D = 64
NS = 128
P = 125          # rows per tile (partition dim)
SC = 48          # max tiles per chunk (DMA + stats granularity)
GT = 4           # tiles per matmul group (4*128 = 512 f32 = 1 PSUM bank)
PSPLIT = 120     # descriptor-count split: 120 (15 engines) + 5

RSQRT_MAGIC = 0x5F3759DF
F32 = mybir.dt.float32
F32R = mybir.dt.float32r
BF16 = mybir.dt.bfloat16
U32 = mybir.dt.uint32


def _split_dma(nc, out_ap, in_ap, eng):
    eng.dma_start(out=out_ap[0:PSPLIT], in_=in_ap[0:PSPLIT])
    eng.dma_start(out=out_ap[PSPLIT:P], in_=in_ap[PSPLIT:P])


def _schedule(ntiles: int) -> list[int]:
    """Graded chunk sizes: small head (fast pipeline fill), 48s in the
    middle, tapered tail (short drain). All sizes are multiples of 4."""
    sched = []
    rem = ntiles
    for s in (8, 8, 16, 32):
        if rem >= s + SC:
            sched.append(s)
            rem -= s
    while rem > SC + 8:
        sched.append(SC)
        rem -= SC
    if rem > 8:
        sched.append(rem - 8)
        sched.append(8)
    elif rem:
        sched.append(rem)
    assert sum(sched) == ntiles, (sched, ntiles)
    assert all(s <= SC and s % 4 == 0 for s in sched), sched
    return sched


def build_program(rows: int):
    """Build the single-core Bass/Tile program for `rows` rows (SPMD across cores)."""
    assert rows % P == 0
    ntiles = rows // P
    assert ntiles % GT == 0

    nc = bacc.Bacc(
        "TRN2",
        target_bir_lowering=False,
        debug=False,
        num_devices=N_CORES,
    )

    x = nc.dram_tensor("x", [rows, D], F32, kind="ExternalInput")
    wmu_d = nc.dram_tensor("wmu", [P, D], F32, kind="ExternalInput")
    elv_d = nc.dram_tensor("elv", [P, D], F32, kind="ExternalInput")
    z2g_d = nc.dram_tensor("z2g", [96, 4 * GT * NS], F32R, kind="ExternalInput")
    ident = nc.dram_tensor("ident", [P, P], F32, kind="ExternalInput")
    # bf16 output: sigmoid values lie in [0, 1], so bf16 costs <1e-3
    # absolute error; the host upcasts to f32 after the gather. This
    # halves the HBM write traffic (the dominant stream).
    out = nc.dram_tensor("out", [rows, NS], BF16, kind="ExternalOutput")

    sched = _schedule(ntiles)
    bases = []
    c0 = 0
    for TC in sched:
        bases.append(c0)
        c0 += TC

    with tile.TileContext(nc) as tc, ExitStack() as ctx:
        singles = ctx.enter_context(tc.tile_pool(name="singles", bufs=1))
        xin = ctx.enter_context(tc.tile_pool(name="xin", bufs=3))
        sqp = ctx.enter_context(tc.tile_pool(name="sqp", bufs=2))
        amp = ctx.enter_context(tc.tile_pool(name="amp", bufs=2))
        statp = ctx.enter_context(tc.tile_pool(name="statp", bufs=3))
        smalls = ctx.enter_context(tc.tile_pool(name="smalls", bufs=3))
        s2p = ctx.enter_context(tc.tile_pool(name="s2p", bufs=3))
        outp = ctx.enter_context(tc.tile_pool(name="outp", bufs=4))
        pst_pool = ctx.enter_context(tc.tile_pool(name="pst", bufs=2, space="PSUM"))
        paff_pool = ctx.enter_context(tc.tile_pool(name="paff", bufs=3, space="PSUM"))

        # chunks are loaded in PAIRS: one DMA covers two consecutive
        # chunks (bigger descriptors, half the issue instructions)
        pairs = []
        i = 0
        while i < len(sched):
            if i + 1 < len(sched):
                pairs.append((i, i + 1))
                i += 2
            else:
                pairs.append((i, None))
                i += 1

        xts = {}

        def issue_pair(pi):
            ca, cb = pairs[pi]
            tw = sched[ca] + (sched[cb] if cb is not None else 0)
            row0 = bases[ca] * P
            xc = x[row0 : row0 + P * tw, :].rearrange("(p t) d -> p t d", p=P)
            # sync ring: E69+ round f32 payloads to f32r there, which
            # costs only ~1e-3 extra output error (host-validated); in
            # exchange the input stream is issued from the otherwise-idle
            # sync sequencer and the scalar ring is free for the bf16
            # outputs (which the sync ring would corrupt by word-masking)
            xt = xin.tile([P, 2 * SC, D], F32, name="xt")
            _split_dma(nc, xt[:, :tw, :], xc, eng=nc.sync)
            # NOTE: the pair is striped as ONE tw-tile chunk (partition p
            # holds rows row0 + p*tw + t); each chunk's output must use
            # the same striping, so carry (row0, tw, toff) along.
            xts[ca] = (xt, 0, row0, tw)
            if cb is not None:
                xts[cb] = (xt, sched[ca], row0, tw)

        issue_pair(0)

        # ---- one-time consts ----
        wmu_p = singles.tile([P, 1, D], F32)
        nc.scalar.dma_start(out=wmu_p[:, 0, :], in_=wmu_d[:, :])
        elv_p = singles.tile([P, 1, D], F32)
        nc.scalar.dma_start(out=elv_p[:, 0, :], in_=elv_d[:, :])
        z2g_sb = singles.tile([96, 4 * GT * NS], F32R)
        nc.sync.dma_start(out=z2g_sb, in_=z2g_d[:, :])
        id_sb = singles.tile([P, P], F32)
        nc.sync.dma_start(out=id_sb, in_=ident[:, :])
        magic_sb = singles.tile([P, SC], U32)
        nc.vector.memset(magic_sb, RSQRT_MAGIC)
        one_sb = singles.tile([P, 1], U32)
        nc.vector.memset(one_sb, 1)

        # ---- chunk pipeline ----
        for pi0 in (1, 2):
            if pi0 < len(pairs):
                issue_pair(pi0)

        first_of_pair = {ca: pi for pi, (ca, cb) in enumerate(pairs)}

        for ci, TC in enumerate(sched):
            xtp, toff, prow0, ptw = xts.pop(ci)
            oc = out[prow0 : prow0 + P * ptw, :].rearrange(
                "(p t) s -> p t s", p=P
            )[:, toff : toff + TC, :]
            xt = xtp[:, toff : toff + TC, :]

            # X^2 on ACT (Square lives in the sigmoid table set)
            x2 = sqp.tile([P, SC, D], F32)
            nc.scalar.activation(
                out=x2[:, :TC, :], in_=xt,
                func=mybir.ActivationFunctionType.Square,
            )
            # prefetch two pairs ahead at each pair's first chunk; the
            # target slot was freed a pair ago, so the dma_start never
            # blocks its ring on the buffer WAR
            pi = first_of_pair.get(ci)
            if pi is not None and pi >= 1 and pi + 2 < len(pairs):
                issue_pair(pi + 2)

            # A = X * w_mu (stride-0 broadcast over t), split ~60/40
            # DVE/GPSIMD; V = X^2 * exp(lv) in place on GPSIMD.
            # Measured rates: DVE big-mul ~77, GPSIMD ~61 G elem/s;
            # reduces+NR put ~6.8us/chunk of fixed work on DVE.
            at = amp.tile([P, SC, D], F32)
            AS = max(4, (3 * TC // 5) & ~3)
            nc.vector.tensor_mul(
                at[:, :AS, :], xt[:, :AS, :], wmu_p.to_broadcast((P, AS, D))
            )
            nc.gpsimd.tensor_mul(
                at[:, AS:TC, :], xt[:, AS:TC, :],
                wmu_p.to_broadcast((P, TC - AS, D)),
            )
            nc.gpsimd.tensor_mul(
                x2[:, :TC, :], x2[:, :TC, :], elv_p.to_broadcast((P, TC, D))
            )

            statblk = statp.tile([P, SC, 2], F32)
            nc.vector.tensor_reduce(
                out=statblk[:, :TC, 0],
                in_=at[:, :TC, :],
                axis=mybir.AxisListType.X,
                op=mybir.AluOpType.add,
            )
            var = smalls.tile([P, SC], F32)
            nc.vector.tensor_reduce(
                out=var[:, :TC],
                in_=x2[:, :TC, :],
                axis=mybir.AxisListType.X,
                op=mybir.AluOpType.add,
            )

            # y = rsqrt(var): seed 0x5f3759df - (bits >> 1), 2 NR iters
            vb = var[:, :TC].bitcast(U32)
            yb = smalls.tile([P, SC], U32)
            nc.vector.tensor_scalar(
                yb[:, :TC], vb, one_sb[:, 0:1], None,
                op0=mybir.AluOpType.logical_shift_right,
            )
            nc.vector.scalar_tensor_tensor(
                out=yb[:, :TC],
                in0=magic_sb[:, :TC],
                scalar=0,
                in1=yb[:, :TC],
                op0=mybir.AluOpType.bypass,
                op1=mybir.AluOpType.subtract,
            )
            y = yb.bitcast(F32)
            t2 = smalls.tile([P, SC], F32)
            for _ in range(2):
                # y <- y*(1.5 - 0.5*var*y^2)
                nc.gpsimd.tensor_mul(t2[:, :TC], y[:, :TC], y[:, :TC])
                nc.vector.scalar_tensor_tensor(
                    out=t2[:, :TC], in0=t2[:, :TC], scalar=-0.5, in1=var[:, :TC],
                    op0=mybir.AluOpType.mult, op1=mybir.AluOpType.mult,
                )
                nc.vector.scalar_tensor_tensor(
                    out=y[:, :TC], in0=t2[:, :TC], scalar=1.5, in1=y[:, :TC],
                    op0=mybir.AluOpType.add, op1=mybir.AluOpType.mult,
                )
            # std -> statblk row 1
            nc.gpsimd.tensor_mul(statblk[:, :TC, 1], var[:, :TC], y[:, :TC])

            # transpose stats: [125, (t k)] -> [(t k), 125]; rows 2t/2t+1
            # hold tile t's mean/std, so group g's rows are partitions
            # [8g, 8g+8)
            pst = pst_pool.tile([2 * SC, P], F32)
            nc.tensor.transpose(
                out=pst[: 2 * TC, :],
                in_=statblk[:, :TC, :].rearrange("p t k -> p (t k)"),
                identity=id_sb,
            )
            s2 = s2p.tile([2 * SC, P], F32R)
            nc.scalar.copy(out=s2[: 2 * TC, :], in_=pst[: 2 * TC, :])

            outb = outp.tile([P, SC, NS], BF16)
            g0 = 0
            while g0 < TC:
                gw = min(2 * GT, TC - g0)          # 8 or tail 4 tiles
                pa = paff_pool.tile([P, 2, GT * NS], F32)
                for k in range(gw // GT):
                    r0 = 2 * (g0 + k * GT)     # stat-row offset of this group
                    b = (r0 // 32) * 32        # legal PE base partition
                    v = (r0 - b) // 8          # which column-variant of z2g
                    kk = min(32, 2 * TC - b)
                    nc.tensor.matmul(
                        pa[:, k, :],
                        lhsT=s2[b : b + kk, :],
                        rhs=z2g_sb[b : b + kk, v * GT * NS : (v + 1) * GT * NS],
                        start=True,
                        stop=True,
                    )
                nc.scalar.activation(
                    out=outb[:, g0 : g0 + gw, :].rearrange("p t s -> p (t s)"),
                    in_=pa.rearrange("p a b -> p (a b)")[:, : gw * NS],
                    func=mybir.ActivationFunctionType.Sigmoid,
                )
                g0 += gw

            _split_dma(nc, oc, outb[:, :TC, :], eng=nc.scalar)

    nc.finalize()
    return nc


def _rn_f32r(x: np.ndarray) -> np.ndarray:
    """Round-to-nearest-even to 11 explicit mantissa bits (f32r)."""
    u = np.ascontiguousarray(x, dtype=np.float32).view(np.uint32)
    add = np.uint32(0x800) - np.uint32(1) + ((u >> np.uint32(12)) & np.uint32(1))
    return ((u + add) & np.uint32(0xFFFFF000)).view(np.float32)


def _host_consts(w_mu: np.ndarray, w_log_var: np.ndarray, z: np.ndarray):
    elv = np.exp(np.asarray(w_log_var, dtype=np.float64)).astype(np.float32)
    z = _rn_f32r(np.asarray(z, dtype=np.float32))
    # [96, 2048]: 3 stacked 32-row blocks (PE base partitions 0/32/64),
    # each with 4 column-variants selecting which 8 rows carry the
    # block-diagonal [ones; z] pattern for a 4-tile matmul group.
    z2g = np.zeros((96, 4 * GT * NS), dtype=np.float32)
    for b in (0, 32, 64):
        for v in range(4):
            for tl in range(GT):
                r = b + 8 * v + 2 * tl
                c = v * GT * NS + tl * NS
                z2g[r, c : c + NS] = 1.0
                z2g[r + 1, c : c + NS] = z
    return {
        "wmu": np.tile(np.asarray(w_mu, dtype=np.float32)[None, :], (P, 1)),
        "elv": np.tile(elv[None, :], (P, 1)),
        "z2g": z2g,
        "ident": np.eye(P, dtype=np.float32),
    }


_PROGRAM_CACHE: dict[int, "bass.Bass"] = {}


def run(X, w_mu, w_log_var, z, trace=False):
    X = np.ascontiguousarray(X, dtype=np.float32)
    n = X.shape[0]
    assert n % N_CORES == 0
    rows = n // N_CORES
    if rows not in _PROGRAM_CACHE:
        _PROGRAM_CACHE[rows] = build_program(rows)
    nc = _PROGRAM_CACHE[rows]

    consts = _host_consts(np.asarray(w_mu), np.asarray(w_log_var), np.asarray(z))
    in_maps = [
        {"x": X[i * rows : (i + 1) * rows], **consts} for i in range(N_CORES)
    ]
    res = run_bass_kernel_spmd(nc, in_maps, list(range(N_CORES)), trace=trace)
    outs = [np.asarray(res.results[i]["out"]).astype(np.float32) for i in range(N_CORES)]
    full = np.concatenate(outs, axis=0)
    return full, res


def kernel(X, w_mu, w_log_var, z):
    full, _ = run(X, w_mu, w_log_var, z, trace=False)
    return full


# revision 48
# speedup vs baseline: 1.2118x; 1.0267x over previous
"""Trainium2 Bass kernel: sampled logistic-regression forward.

reference math (per data row i, sample s):
    mean_i = X[i] . w_mu
    var_i  = sum_d X[i,d]^2 * exp(w_log_var[d])
    out[i,s] = sigmoid( sqrt(var_i) * z[s] + mean_i )

Full shapes: X [500000, 64], w_mu [64], w_log_var [64], z [128]
Output: [500000, 128] fp32.

Sharding: data-parallel over 8 NeuronCores, 62500 rows each.

Layout: chunk-local stripe. Chunk c covers shard rows
[base, base + 125*TC) -- one contiguous DRAM range per DMA -- and
within the chunk, partition p holds rows base + p*TC + t. Each DMA
descriptor is a per-partition contiguous run of TC rows and each DMA
instruction's DRAM side is one contiguous range. DMAs split into a
120-descriptor + 5-descriptor instruction pair: descriptor counts
divisible by 15 fan out across all 15 DMA engines.

Numerics: tolerance is rel 2e-2; measured full-size error is ~5.4e-3:
  - stats are plain f32r (11-bit mantissa) [mean, std] rows (K=2), no
    hi/lo splitting or mantissa masks;
  - X rides the sync ring, whose E69+ DMA engines round f32 payloads
    to f32r (~+1e-3);
  - the output is written to DRAM as bf16 (sigmoid in [0,1] -> <1e-3
    absolute) and upcast to f32 on the host, halving write traffic.
All three simplifications were validated against the float64 reference
on the actual seed-0 data before being adopted.

Per-core pipeline, chunks of up to SC=48 tiles x [125 rows, 64],
loaded in PAIRS (one DMA covers two chunks: 24 KB descriptors, half
the issue instructions; outputs use the pair's tw-stripe):
  - in-DMA pair (sync ring, issued 2 pairs ahead from the otherwise
    idle sync sequencer; target slot freed a pair ago so the issue
    never parks the ring on a buffer WAR)
  - ACT: X2 = Square(X) (Square lives in the sigmoid table set)
  - DVE/GPSIMD: A = X * w_mu split ~60/40 (stride-0 broadcast of the
    [P, 1, D] weight rows -- measured same speed as materialized reps);
    V = X2 * exp(lv) in place on GPSIMD
  - DVE: reduce A -> statblk[:, :, 0] (mean); reduce V -> var
  - DVE+GPSIMD: Newton rsqrt (bit-trick seed, 2 iters);
    statblk[:, :, 1] = var * y (std)
  - PE: transpose statblk [125, (t k)] -> [2*TC, 125]; ACT copy ->
    SBUF f32r (rows 2t/2t+1 = tile t's mean/std, so each 4-tile
    group's 8 stat rows are contiguous partitions)
  - PE: per 4-tile group one matmul lhsT=s2[b:b+K] (K<=32, base
    partition b in {0,32,64}) x rhs=z2g[b:b+K, 512-col variant] where
    z2g [96, 2048] f32r holds 3 stacked 32-row blocks x 4 column
    variants of the block-diagonal [ones; z] pattern
  - ACT: Sigmoid [125, 1024] over PSUM bank pairs -> outb (bf16)
  - out-DMA full chunk (scalar ring -- the sync ring would corrupt
    bf16 payloads by f32r word-masking)
"""

from contextlib import ExitStack

import numpy as np

import concourse.bacc as bacc
import concourse.bass as bass
import concourse.tile as tile
from concourse import mybir
from concourse.bass_utils import run_bass_kernel_spmd

N_CORES = 8
D = 64
NS = 128
P = 125          # rows per tile (partition dim)
SC = 48          # max tiles per chunk (DMA + stats granularity)
GT = 4           # tiles per matmul group (4*128 = 512 f32 = 1 PSUM bank)
PSPLIT = 120     # descriptor-count split: 120 (15 engines) + 5

RSQRT_MAGIC = 0x5F3759DF
F32 = mybir.dt.float32
F32R = mybir.dt.float32r
BF16 = mybir.dt.bfloat16
U32 = mybir.dt.uint32


def _split_dma(nc, out_ap, in_ap, eng):
    eng.dma_start(out=out_ap[0:PSPLIT], in_=in_ap[0:PSPLIT])
    eng.dma_start(out=out_ap[PSPLIT:P], in_=in_ap[PSPLIT:P])


def _schedule(ntiles: int) -> list[int]:
    """Graded chunk sizes: small head (fast pipeline fill), 48s in the
    middle, tapered tail (short drain). All sizes are multiples of 4."""
    sched = []
    rem = ntiles
    for s in (8, 8, 16, 32):
        if rem >= s + SC:
            sched.append(s)
            rem -= s
    while rem > SC + 8:
        sched.append(SC)
        rem -= SC
    if rem > 8:
        sched.append(rem - 8)
        sched.append(8)
    elif rem:
        sched.append(rem)
    assert sum(sched) == ntiles, (sched, ntiles)
    assert all(s <= SC and s % 4 == 0 for s in sched), sched
    return sched


def build_program(rows: int):
    """Build the single-core Bass/Tile program for `rows` rows (SPMD across cores)."""
    assert rows % P == 0
    ntiles = rows // P
    assert ntiles % GT == 0

    nc = bacc.Bacc(
        "TRN2",
        target_bir_lowering=False,
        debug=False,
        num_devices=N_CORES,
    )

    x = nc.dram_tensor("x", [rows, D], F32, kind="ExternalInput")
    wmu_d = nc.dram_tensor("wmu", [P, D], F32, kind="ExternalInput")
    elv_d = nc.dram_tensor("elv", [P, D], F32, kind="ExternalInput")
    z2g_d = nc.dram_tensor("z2g", [96, 4 * GT * NS], F32R, kind="ExternalInput")
    ident = nc.dram_tensor("ident", [P, P], F32, kind="ExternalInput")
    # bf16 output: sigmoid values lie in [0, 1], so bf16 costs <1e-3
    # absolute error; the host upcasts to f32 after the gather. This
    # halves the HBM write traffic (the dominant stream).
    out = nc.dram_tensor("out", [rows, NS], BF16, kind="ExternalOutput")

    sched = _schedule(ntiles)
    bases = []
    c0 = 0
    for TC in sched:
        bases.append(c0)
        c0 += TC

    with tile.TileContext(nc) as tc, ExitStack() as ctx:
        singles = ctx.enter_context(tc.tile_pool(name="singles", bufs=1))
        xin = ctx.enter_context(tc.tile_pool(name="xin", bufs=3))
        sqp = ctx.enter_context(tc.tile_pool(name="sqp", bufs=2))
        amp = ctx.enter_context(tc.tile_pool(name="amp", bufs=2))
        statp = ctx.enter_context(tc.tile_pool(name="statp", bufs=3))
        smalls = ctx.enter_context(tc.tile_pool(name="smalls", bufs=3))
        s2p = ctx.enter_context(tc.tile_pool(name="s2p", bufs=3))
        outp = ctx.enter_context(tc.tile_pool(name="outp", bufs=4))
        pst_pool = ctx.enter_context(tc.tile_pool(name="pst", bufs=2, space="PSUM"))
        paff_pool = ctx.enter_context(tc.tile_pool(name="paff", bufs=3, space="PSUM"))

        # chunks are loaded in PAIRS: one DMA covers two consecutive
        # chunks (bigger descriptors, half the issue instructions)
        pairs = []
        i = 0
        while i < len(sched):
            if i + 1 < len(sched):
                pairs.append((i, i + 1))
                i += 2
            else:
                pairs.append((i, None))
                i += 1

        xts = {}

        def issue_pair(pi):
            ca, cb = pairs[pi]
            tw = sched[ca] + (sched[cb] if cb is not None else 0)
            row0 = bases[ca] * P
            xc = x[row0 : row0 + P * tw, :].rearrange("(p t) d -> p t d", p=P)
            # sync ring: E69+ round f32 payloads to f32r there, which
            # costs only ~1e-3 extra output error (host-validated); in
            # exchange the input stream is issued from the otherwise-idle
            # sync sequencer and the scalar ring is free for the bf16
            # outputs (which the sync ring would corrupt by word-masking)
            xt = xin.tile([P, 2 * SC, D], F32, name="xt")
            _split_dma(nc, xt[:, :tw, :], xc, eng=nc.sync)
            # NOTE: the pair is striped as ONE tw-tile chunk (partition p
            # holds rows row0 + p*tw + t); each chunk's output must use
            # the same striping, so carry (row0, tw, toff) along.
            xts[ca] = (xt, 0, row0, tw)
            if cb is not None:
                xts[cb] = (xt, sched[ca], row0, tw)

        issue_pair(0)

        # ---- one-time consts ----
        wmu_p = singles.tile([P, 1, D], F32)
        nc.scalar.dma_start(out=wmu_p[:, 0, :], in_=wmu_d[:, :])
        elv_p = singles.tile([P, 1, D], F32)
        nc.scalar.dma_start(out=elv_p[:, 0, :], in_=elv_d[:, :])
        z2g_sb = singles.tile([96, 4 * GT * NS], F32R)
        nc.sync.dma_start(out=z2g_sb, in_=z2g_d[:, :])
        id_sb = singles.tile([P, P], F32)
        nc.sync.dma_start(out=id_sb, in_=ident[:, :])
        magic_sb = singles.tile([P, SC], U32)
        nc.vector.memset(magic_sb, RSQRT_MAGIC)
        one_sb = singles.tile([P, 1], U32)
        nc.vector.memset(one_sb, 1)

        # ---- chunk pipeline ----
        for pi0 in (1, 2):
            if pi0 < len(pairs):
                issue_pair(pi0)

        first_of_pair = {ca: pi for pi, (ca, cb) in enumerate(pairs)}

        for ci, TC in enumerate(sched):
            xtp, toff, prow0, ptw = xts.pop(ci)
            oc = out[prow0 : prow0 + P * ptw, :].rearrange(
                "(p t) s -> p t s", p=P
            )[:, toff : toff + TC, :]
            xt = xtp[:, toff : toff + TC, :]

            # X^2 on ACT (Square lives in the sigmoid table set)
            x2 = sqp.tile([P, SC, D], F32)
            nc.scalar.activation(
                out=x2[:, :TC, :], in_=xt,
                func=mybir.ActivationFunctionType.Square,
            )
            # prefetch two pairs ahead at each pair's first chunk; the
            # target slot was freed a pair ago, so the dma_start never
            # blocks its ring on the buffer WAR
            pi = first_of_pair.get(ci)
            if pi is not None and pi >= 1 and pi + 2 < len(pairs):
                issue_pair(pi + 2)

            # A = X * w_mu (stride-0 broadcast over t), split ~60/40
            # DVE/GPSIMD; V = X^2 * exp(lv) in place on GPSIMD.
            # Measured rates: DVE big-mul ~77, GPSIMD ~61 G elem/s;
            # reduces+NR put ~6.8us/chunk of fixed work on DVE.
            at = amp.tile([P, SC, D], F32)
            AS = max(4, (3 * TC // 5) & ~3)
            nc.vector.tensor_mul(
                at[:, :AS, :], xt[:, :AS, :], wmu_p.to_broadcast((P, AS, D))
            )
            nc.gpsimd.tensor_mul(
                at[:, AS:TC, :], xt[:, AS:TC, :],
                wmu_p.to_broadcast((P, TC - AS, D)),
            )
            # V-mul in halves so each half's reduce overlaps the other
            # half's mul (the mul->reduce chain is the cycle's long pole)
            VH = max(4, (TC // 2) & ~3)
            nc.gpsimd.tensor_mul(
                x2[:, :VH, :], x2[:, :VH, :], elv_p.to_broadcast((P, VH, D))
            )

            statblk = statp.tile([P, SC, 2], F32)
            # mean over the DVE-computed part is ready first
            nc.vector.tensor_reduce(
                out=statblk[:, :AS, 0],
                in_=at[:, :AS, :],
                axis=mybir.AxisListType.X,
                op=mybir.AluOpType.add,
            )
            var = smalls.tile([P, SC], F32)
            nc.vector.tensor_reduce(
                out=var[:, :VH],
                in_=x2[:, :VH, :],
                axis=mybir.AxisListType.X,
                op=mybir.AluOpType.add,
            )
            if VH < TC:
                nc.gpsimd.tensor_mul(
                    x2[:, VH:TC, :], x2[:, VH:TC, :],
                    elv_p.to_broadcast((P, TC - VH, D)),
                )
                nc.vector.tensor_reduce(
                    out=var[:, VH:TC],
                    in_=x2[:, VH:TC, :],
                    axis=mybir.AxisListType.X,
                    op=mybir.AluOpType.add,
                )
            if AS < TC:
                nc.vector.tensor_reduce(
                    out=statblk[:, AS:TC, 0],
                    in_=at[:, AS:TC, :],
                    axis=mybir.AxisListType.X,
                    op=mybir.AluOpType.add,
                )

            # y = rsqrt(var): seed 0x5f3759df - (bits >> 1), 2 NR iters
            vb = var[:, :TC].bitcast(U32)
            yb = smalls.tile([P, SC], U32)
            nc.vector.tensor_scalar(
                yb[:, :TC], vb, one_sb[:, 0:1], None,
                op0=mybir.AluOpType.logical_shift_right,
            )
            nc.vector.scalar_tensor_tensor(
                out=yb[:, :TC],
                in0=magic_sb[:, :TC],
                scalar=0,
                in1=yb[:, :TC],
                op0=mybir.AluOpType.bypass,
                op1=mybir.AluOpType.subtract,
            )
            y = yb.bitcast(F32)
            t2 = smalls.tile([P, SC], F32)
            for _ in range(2):
                # y <- y*(1.5 - 0.5*var*y^2)
                nc.gpsimd.tensor_mul(t2[:, :TC], y[:, :TC], y[:, :TC])
                nc.vector.scalar_tensor_tensor(
                    out=t2[:, :TC], in0=t2[:, :TC], scalar=-0.5, in1=var[:, :TC],
                    op0=mybir.AluOpType.mult, op1=mybir.AluOpType.mult,
                )
                nc.vector.scalar_tensor_tensor(
                    out=y[:, :TC], in0=t2[:, :TC], scalar=1.5, in1=y[:, :TC],
                    op0=mybir.AluOpType.add, op1=mybir.AluOpType.mult,
                )
            # std -> statblk row 1
            nc.gpsimd.tensor_mul(statblk[:, :TC, 1], var[:, :TC], y[:, :TC])

            # transpose stats: [125, (t k)] -> [(t k), 125]; rows 2t/2t+1
            # hold tile t's mean/std, so group g's rows are partitions
            # [8g, 8g+8)
            pst = pst_pool.tile([2 * SC, P], F32)
            nc.tensor.transpose(
                out=pst[: 2 * TC, :],
                in_=statblk[:, :TC, :].rearrange("p t k -> p (t k)"),
                identity=id_sb,
            )
            s2 = s2p.tile([2 * SC, P], F32R)
            nc.scalar.copy(out=s2[: 2 * TC, :], in_=pst[: 2 * TC, :])

            outb = outp.tile([P, SC, NS], BF16)
            g0 = 0
            while g0 < TC:
                gw = min(2 * GT, TC - g0)          # 8 or tail 4 tiles
                pa = paff_pool.tile([P, 2, GT * NS], F32)
                for k in range(gw // GT):
                    r0 = 2 * (g0 + k * GT)     # stat-row offset of this group
                    b = (r0 // 32) * 32        # legal PE base partition
                    v = (r0 - b) // 8          # which column-variant of z2g
                    kk = min(32, 2 * TC - b)
                    nc.tensor.matmul(
                        pa[:, k, :],
                        lhsT=s2[b : b + kk, :],
                        rhs=z2g_sb[b : b + kk, v * GT * NS : (v + 1) * GT * NS],
                        start=True,
                        stop=True,
                    )
                nc.scalar.activation(
                    out=outb[:, g0 : g0 + gw, :].rearrange("p t s -> p (t s)"),
                    in_=pa.rearrange("p a b -> p (a b)")[:, : gw * NS],
                    func=mybir.ActivationFunctionType.Sigmoid,
                )
                g0 += gw

            _split_dma(nc, oc, outb[:, :TC, :], eng=nc.scalar)

    nc.finalize()
    return nc


def _rn_f32r(x: np.ndarray) -> np.ndarray:
    """Round-to-nearest-even to 11 explicit mantissa bits (f32r)."""
    u = np.ascontiguousarray(x, dtype=np.float32).view(np.uint32)
    add = np.uint32(0x800) - np.uint32(1) + ((u >> np.uint32(12)) & np.uint32(1))
    return ((u + add) & np.uint32(0xFFFFF000)).view(np.float32)


def _host_consts(w_mu: np.ndarray, w_log_var: np.ndarray, z: np.ndarray):
    elv = np.exp(np.asarray(w_log_var, dtype=np.float64)).astype(np.float32)
    z = _rn_f32r(np.asarray(z, dtype=np.float32))
    # [96, 2048]: 3 stacked 32-row blocks (PE base partitions 0/32/64),
    # each with 4 column-variants selecting which 8 rows carry the
    # block-diagonal [ones; z] pattern for a 4-tile matmul group.
    z2g = np.zeros((96, 4 * GT * NS), dtype=np.float32)
    for b in (0, 32, 64):
        for v in range(4):
            for tl in range(GT):
                r = b + 8 * v + 2 * tl
                c = v * GT * NS + tl * NS
                z2g[r, c : c + NS] = 1.0
                z2g[r + 1, c : c + NS] = z
    return {
        "wmu": np.tile(np.asarray(w_mu, dtype=np.float32)[None, :], (P, 1)),
        "elv": np.tile(elv[None, :], (P, 1)),
        "z2g": z2g,
        "ident": np.eye(P, dtype=np.float32),
    }


_PROGRAM_CACHE: dict[int, "bass.Bass"] = {}


def run(X, w_mu, w_log_var, z, trace=False):
    X = np.ascontiguousarray(X, dtype=np.float32)
    n = X.shape[0]
    assert n % N_CORES == 0
    rows = n // N_CORES
    if rows not in _PROGRAM_CACHE:
        _PROGRAM_CACHE[rows] = build_program(rows)
    nc = _PROGRAM_CACHE[rows]

    consts = _host_consts(np.asarray(w_mu), np.asarray(w_log_var), np.asarray(z))
    in_maps = [
        {"x": X[i * rows : (i + 1) * rows], **consts} for i in range(N_CORES)
    ]
    res = run_bass_kernel_spmd(nc, in_maps, list(range(N_CORES)), trace=trace)
    outs = [np.asarray(res.results[i]["out"]).astype(np.float32) for i in range(N_CORES)]
    full = np.concatenate(outs, axis=0)
    return full, res


def kernel(X, w_mu, w_log_var, z):
    full, _ = run(X, w_mu, w_log_var, z, trace=False)
    return full
